# revision 28
# baseline (speedup 1.0000x reference)
"""Trainium2 8-core GQA causal attention kernel (Bass/Tile).

Problem: B=2, S=2048, D=4096, 32 Q heads / 8 KV heads, HD=128, RoPE
(interleaved pairs), causal mask, output projection.

Sharding: 8-way tensor parallel over KV-head groups. Core i owns query
heads 4i..4i+3 (wq cols i*512..), kv head i (wk/wv cols i*128..), and
OUTPUT columns i*512.. of wo.  Per core (all in transposed layout; the
host passes xT and tile-packed weights so every DMA is contiguous):
  qT = wq_i.T @ x.T ; kT = wk_i.T @ x.T ; vT = wv_i.T @ x.T
  RoPE: z*cosF + (Pswap z)*sinF2  (pair swap via PE permutation matmul)
  v -> token-major via PE transpose
  S^T[tk,tq] = kT_tile.T @ qT_chunk (+mask on diagonal blocks)
  es = exp(S^T * scale) fused on ScalarE (PSUM->SBUF)
  out^T[c,tq] += v_tile @ es ; Z[tq] += ones @ es (replicated col-sums)
  out^T = out^T / Z  -> outT chunk (bf16)
AllGather (4 token-range chunks, overlapped with attention) -> attnT;
outP = wo_i.T @ attnT  (512 out cols, T); host concatenates + transposes.
"""
import sys
import numpy as np

sys.path.insert(0, "/opt/trn_rl_repo")

import ml_dtypes  # noqa: E402

BF16 = ml_dtypes.bfloat16

NCORES = 8
B, S, D = 2, 2048, 4096
H, KV, HD = 32, 8, 128
T = B * S
HPC = H // NCORES          # 4 query heads per core
CQ = HPC * HD              # 512
CKV = HD                   # 128
SC = 512                   # token chunk (free dim of moving operands)
ND = D // 128              # 32 contraction chunks
NT = T // SC               # 8 token chunks
NA = NCORES * CQ // 128    # 32 attention-dim chunks in phase 3
NAG = 4                    # all-gather chunks (2 token chunks each)
SCALE = float(HD) ** -0.5


def _pack(a, width):
    """(n*128, width) -> (n, 128*width) tile-contiguous rows."""
    n = a.shape[0] // 128
    return np.ascontiguousarray(a.reshape(n, 128, width).reshape(n, 128 * width))


def host_prepare(x, cos, sin, mask, wq, wk, wv, wo):
    xM = np.ascontiguousarray(np.asarray(x, dtype=np.float32).reshape(T, D))
    xT = np.ascontiguousarray(xM.T).astype(BF16)                 # (D, T)
    # xTp[d*NT+t] = tile (d-chunk, t-chunk) flattened (128, SC)
    xTp = np.ascontiguousarray(
        xT.reshape(ND, 128, NT, SC).transpose(0, 2, 1, 3)
    ).reshape(ND * NT, 128 * SC)
    cosF = np.repeat(np.asarray(cos, dtype=np.float32).T, 2, axis=0).astype(BF16)
    sinF2 = np.repeat(np.asarray(sin, dtype=np.float32).T, 2, axis=0)
    sinF2[0::2] *= -1.0
    sinF2 = sinF2.astype(BF16)                                   # (128, S)
    pswap = np.zeros((128, 128), dtype=np.float32)
    idx = np.arange(0, 128, 2)
    pswap[idx, idx + 1] = 1.0
    pswap[idx + 1, idx] = 1.0
    pswapT = pswap.astype(BF16)
    ident = np.eye(128, dtype=np.float32).astype(BF16)
    ones = np.ones((128, 128), dtype=np.float32).astype(BF16)
    # 0/1 keep-mask (bf16), applied multiplicatively AFTER exp
    maskT4 = np.concatenate(
        [(np.asarray(mask, dtype=np.float32)[0:SC, r * 128:(r + 1) * 128].T
          == 0.0).astype(np.float32)
         for r in range(4)], axis=1
    ).astype(BF16)                                               # (128, 4*SC)
    shared = dict(xT=xTp, cosF=cosF, sinF2=sinF2, pswapT=pswapT, ident=ident,
                  ones=ones, maskT4=maskT4)
    cores = []
    for i in range(NCORES):
        cores.append(dict(
            wq=_pack(np.ascontiguousarray(wq[:, i * CQ:(i + 1) * CQ]).astype(BF16), CQ),
            wk=_pack(np.ascontiguousarray(wk[:, i * CKV:(i + 1) * CKV]).astype(BF16), CKV),
            wv=_pack(np.ascontiguousarray(wv[:, i * CKV:(i + 1) * CKV]).astype(BF16), CKV),
            wo=_pack(np.ascontiguousarray(wo[:, i * CQ:(i + 1) * CQ]).astype(BF16), CQ),
        ))
    return shared, cores


_CACHE = {}


def build_nc():
    from concourse import bacc, mybir, tile

    F32 = mybir.dt.float32
    CDT = mybir.dt.bfloat16
    ADD = mybir.AluOpType.add
    DIV = mybir.AluOpType.divide
    MULT = mybir.AluOpType.mult
    EXP = mybir.ActivationFunctionType.Exp
    COPY = mybir.ActivationFunctionType.Copy

    nc = bacc.Bacc("TRN2", target_bir_lowering=False, debug=False,
                   num_devices=NCORES)

    def par(name, shape, dt, out=False):
        return nc.dram_tensor(name, shape, dt,
                              kind="ExternalOutput" if out else "ExternalInput").ap()

    xT_p = par("xT", [ND * NT, 128 * SC], CDT)
    wq_p = par("wq", [ND, 128 * CQ], CDT)
    wk_p = par("wk", [ND, 128 * CKV], CDT)
    wv_p = par("wv", [ND, 128 * CKV], CDT)
    wo_p = par("wo", [ND, 128 * CQ], CDT)
    cos_p = par("cosF", [HD, S], CDT)
    sin_p = par("sinF2", [HD, S], CDT)
    psw_p = par("pswapT", [128, 128], CDT)
    idn_p = par("ident", [128, 128], CDT)
    one_p = par("ones", [128, 128], CDT)
    msk_p = par("maskT4", [128, 4 * SC], CDT)
    # output: outP[c, t] packed as [(c/128)*NT + t-chunk, 128*SC]
    out_p = par("out", [(CQ // 128) * NT, 128 * SC], F32, out=True)

    xT_t = xT_p.rearrange("n (p c) -> n p c", p=128)
    out_t = out_p.rearrange("n (p c) -> n p c", p=128)

    with tile.TileContext(nc) as tc:
        with tc.tile_pool(name="res", bufs=1) as res, \
             tc.tile_pool(name="dram", bufs=1, space="DRAM") as dram:
            kT = res.tile([128, T], CDT, tag="kT")
            vsb = res.tile([128, T], CDT, tag="vsb")
            qT = res.tile([128, HPC * T], CDT, tag="qT")
            mk = res.tile([128, 4 * SC], CDT, tag="mk")
            ones = res.tile([128, 128], CDT, tag="ones")
            wos = res.tile([128, ND * CQ], CDT, tag="wos")
            nc.sync.dma_start(out=ones[:], in_=one_p[:])
            # packed outT rows: row = g*HPC + h
            outT_d = dram.tile([NT * HPC, 128 * SC], CDT, tag="outT")
            ag_d = [dram.tile([NCORES * HPC, 128 * SC], CDT, tag=f"ag{g}",
                              name=f"ag{g}", addr_space="Shared")
                    for g in range(NT)]

            # ---------------- phase 1: projections + rope + v transpose
            with tc.tile_pool(name="p1c", bufs=1) as p1c, \
                 tc.tile_pool(name="p1x", bufs=4) as p1x, \
                 tc.tile_pool(name="p1s", bufs=3) as p1s, \
                 tc.tile_pool(name="p1r", bufs=7) as p1r, \
                 tc.tile_pool(name="ps1", bufs=1, space="PSUM") as ps1, \
                 tc.tile_pool(name="ps1b", bufs=2, space="PSUM") as ps1b:
                wqs = p1c.tile([128, ND * CQ], CDT, tag="wqs")
                wks = p1c.tile([128, ND * CKV], CDT, tag="wks")
                wvs = p1c.tile([128, ND * CKV], CDT, tag="wvs")
                cosF = p1c.tile([128, S], CDT, tag="cosF")
                sinF2 = p1c.tile([128, S], CDT, tag="sinF2")
                psw = p1c.tile([128, 128], CDT, tag="psw")
                idn = p1c.tile([128, 128], CDT, tag="idn")
                nc.sync.dma_start(out=psw[:], in_=psw_p[:])
                nc.sync.dma_start(out=idn[:], in_=idn_p[:])
                # weights on the gpsimd queue so they don't block x tiles;
                # interleaved by d-chunk so d=0 of all three lands first.
                wq_t = wq_p.rearrange("n (p c) -> n p c", p=128)
                wk_t = wk_p.rearrange("n (p c) -> n p c", p=128)
                wv_t = wv_p.rearrange("n (p c) -> n p c", p=128)
                wo_t = wo_p.rearrange("n (p c) -> n p c", p=128)
                for d in range(ND):
                    nc.gpsimd.dma_start(out=wqs[:, d * CQ:(d + 1) * CQ],
                                        in_=wq_t[d])
                    nc.gpsimd.dma_start(out=wks[:, d * CKV:(d + 1) * CKV],
                                        in_=wk_t[d])
                    nc.gpsimd.dma_start(out=wvs[:, d * CKV:(d + 1) * CKV],
                                        in_=wv_t[d])
                    if d == 0:
                        nc.gpsimd.dma_start(out=cosF[:], in_=cos_p[:])
                        nc.gpsimd.dma_start(out=sinF2[:], in_=sin_p[:])
                # mask (phase 2) then wo (phase 3) trickle in behind the
                # projection weights on the gpsimd queue.
                nc.gpsimd.dma_start(out=mk[:], in_=msk_p[:])
                for d in range(ND):
                    nc.gpsimd.dma_start(out=wos[:, d * CQ:(d + 1) * CQ],
                                        in_=wo_t[d])

                for tcn in range(NT):
                    t0 = tcn * SC
                    s0 = (tcn % (S // SC)) * SC
                    pq = [ps1.tile([128, SC], F32, tag=f"pq{h}", name=f"pq{h}")
                          for h in range(HPC)]
                    pk = ps1.tile([128, SC], F32, tag="pk")
                    pv = ps1.tile([128, SC], F32, tag="pv")
                    for d in range(ND):
                        xt = p1x.tile([128, SC], CDT, tag="xt")
                        nc.sync.dma_start(out=xt[:], in_=xT_t[d * NT + tcn])
                        st = (d == 0)
                        sp = (d == ND - 1)
                        for h in range(HPC):
                            nc.tensor.matmul(
                                pq[h][:],
                                wqs[:, d * CQ + h * 128:d * CQ + (h + 1) * 128],
                                xt[:], start=st, stop=sp)
                        nc.tensor.matmul(pk[:],
                                         wks[:, d * CKV:(d + 1) * CKV],
                                         xt[:], start=st, stop=sp)
                        nc.tensor.matmul(pv[:],
                                         wvs[:, d * CKV:(d + 1) * CKV],
                                         xt[:], start=st, stop=sp)

                    # evict all psums first (frees banks for next chunk);
                    # alternate ScalarE/VectorE so the eviction chain drains
                    # at twice the single-engine rate (RoPE psh matmuls wait
                    # on these).
                    raws = []
                    for z in range(HPC + 2):
                        src = pq[z] if z < HPC else (pk if z == HPC else pv)
                        raw = p1r.tile([128, SC], CDT, tag="raw",
                                       name=f"raw{z}")
                        if z % 2 == 0:
                            nc.scalar.activation(raw[:], src[:], COPY)
                        else:
                            nc.vector.tensor_scalar_add(raw[:], src[:], 0.0)
                        raws.append(raw)
                    # rope (q heads + k)
                    for z in range(HPC + 1):
                        raw = raws[z]
                        psh = ps1b.tile([128, SC], F32, tag="psx", name="psh")
                        nc.tensor.matmul(psh[:], psw[:], raw[:],
                                         start=True, stop=True)
                        t1 = p1s.tile([128, SC], CDT, tag="t1")
                        nc.vector.tensor_tensor(t1[:], raw[:],
                                                cosF[:, s0:s0 + SC], MULT)
                        t2 = p1s.tile([128, SC], CDT, tag="t2")
                        nc.vector.tensor_tensor(t2[:], psh[:],
                                                sinF2[:, s0:s0 + SC], MULT)
                        if z < HPC:
                            dst = qT[:, z * T + t0:z * T + t0 + SC]
                        else:
                            dst = kT[:, t0:t0 + SC]
                        nc.vector.tensor_tensor(dst, t1[:], t2[:], ADD)
                    # v -> token-major via PE transpose
                    vts = raws[HPC + 1]
                    for j in range(SC // 128):
                        ptr = ps1b.tile([128, 128], CDT, tag="psx", name="ptr")
                        nc.tensor.transpose(ptr[:],
                                            vts[:, j * 128:(j + 1) * 128],
                                            idn[:])
                        nc.scalar.activation(
                            vsb[:, t0 + j * 128:t0 + (j + 1) * 128],
                            ptr[:], COPY)

            # ---------------- phase 2+3: attention + interleaved out-proj
            # Head-pair-wide layout: scores/es/po tiles cover 2 heads
            # ([128, 2*SC]). Z comes from a VectorE-accumulated esum plus a
            # single ones-matmul per head. The causal mask is applied
            # multiplicatively (0/1, bf16) AFTER exp on GpSimd, keeping the
            # scores->exp chain a pure PE->ScalarE path. Diagonal blocks are
            # trimmed: key block r only touches queries >= 128*r.
            # Phase-3 (out-projection) work is consumed as PE filler inside
            # attention once its AllGather chunk has landed, then drained.
            outT_t = outT_d.rearrange("n (p c) -> n p c", p=128)
            ags = [d.rearrange("n (p c) -> n p c", p=128) for d in ag_d]
            with tc.tile_pool(name="p2s", bufs=8) as p2s, \
                 tc.tile_pool(name="p2e", bufs=2) as p2e, \
                 tc.tile_pool(name="p2z", bufs=2) as p2z, \
                 tc.tile_pool(name="p3a", bufs=12) as p3a, \
                 tc.tile_pool(name="p3o", bufs=3) as p3o, \
                 tc.tile_pool(name="ps3f", bufs=1, space="PSUM") as ps3f:
                # phase-3 units: (g, half, a); two out-col blocks per half.
                p3_units = [(gp, half, a) for gp in range(NT)
                            for half in range(2) for a in range(NA)]
                p3_state = dict(cursor=0, po3=None)
                # simulated-time anchors (ms): start-of-block estimates used
                # to stop the scheduler from hoisting filler work ahead of
                # its AllGather (hoisted units stall the in-order PE/sync
                # queues on the unfinished collective).
                simt = [0.40]
                for g2 in range(NT):
                    qj2 = g2 % (S // SC)
                    simt.append(simt[-1] + 2 * 4 * (qj2 + 1) * 0.0011
                                + 0.004)

                def p3_eligible(gcur):
                    if p3_state["cursor"] >= len(p3_units):
                        return False
                    # wide margin: a filler matmul stuck on a late
                    # AllGather would block the whole PE stream
                    return gcur >= p3_units[p3_state["cursor"]][0] + 4

                def emit_p3_unit():
                    gp, half, a = p3_units[p3_state["cursor"]]
                    with tc.tile_wait_until(simt[min(gp + 3, NT)]):
                        _emit_p3_unit_inner(gp, half, a)
                    p3_state["cursor"] += 1

                def _emit_p3_unit_inner(gp, half, a):
                    if a == 0:
                        pool = p3_state.get("pool") or ps3f
                        p3_state["po3"] = pool.tile([128, 2 * SC], F32,
                                                    tag="po3", name="po3")
                    po3 = p3_state["po3"]
                    core, h = divmod(a, HPC)
                    at = p3a.tile([128, SC], CDT, tag="at")
                    nc.sync.dma_start(out=at[:], in_=ags[gp][core * HPC + h])
                    st = (a == 0)
                    sp = (a == NA - 1)
                    for j in range(2):
                        c = 2 * half + j
                        nc.tensor.matmul(
                            po3[:, j * SC:(j + 1) * SC],
                            wos[:, a * CQ + c * 128:a * CQ + (c + 1) * 128],
                            at[:], start=st, stop=sp)
                    if a == NA - 1:
                        # evict the two out-col blocks, split across
                        # ScalarE/VectorE to balance engine load
                        ob = p3o.tile([128, 2 * SC], F32, tag="ob")
                        nc.scalar.activation(ob[:, 0:SC], po3[:, 0:SC],
                                             COPY)
                        nc.vector.tensor_scalar_add(ob[:, SC:2 * SC],
                                                    po3[:, SC:2 * SC], 0.0)
                        for j in range(2):
                            c = 2 * half + j
                            nc.sync.dma_start(out=out_t[c * NT + gp],
                                              in_=ob[:, j * SC:(j + 1) * SC])

                ps2ctx = [tc.tile_pool(name="ps2s", bufs=2, space="PSUM"),
                          tc.tile_pool(name="ps2o", bufs=1, space="PSUM")]
                ps2s = ps2ctx[0].__enter__()
                ps2o = ps2ctx[1].__enter__()
                for g in range(NT):
                    b, qj = divmod(g, S // SC)
                    nkb = (SC // 128) * (qj + 1)
                    for hp in range(HPC // 2):
                        h0 = 2 * hp
                        po = ps2o.tile([128, 2 * SC], F32, tag="po",
                                       name="po")
                        esum = p2e.tile([128, 2 * SC], CDT, tag="esum",
                                        name="esum")
                        ess = []

                        def emit_pv(kb):
                            es, tkb, off = ess[kb]
                            st = (kb == 0)
                            sp = (kb == nkb - 1)
                            for hh in range(2):
                                nc.tensor.matmul(
                                    po[:, hh * SC + off:(hh + 1) * SC],
                                    vsb[:, tkb:tkb + 128],
                                    es[:, hh * SC + off:(hh + 1) * SC],
                                    start=st, stop=sp)

                        def emit_esum(kb):
                            # trails one block behind exp so the DVE never
                            # head-of-line blocks on exp(kb)
                            es, tkb, off = ess[kb]
                            if kb == 0:
                                nc.vector.tensor_scalar_add(
                                    esum[:], es[:], 0.0)
                            else:
                                es_ap = (es[:].rearrange(
                                    "p (h q) -> p h q", h=2)[:, :, off:]
                                    if off else es[:])
                                em_ap = (esum[:].rearrange(
                                    "p (h q) -> p h q", h=2)[:, :, off:]
                                    if off else esum[:])
                                nc.vector.tensor_tensor(
                                    em_ap, em_ap, es_ap, ADD)

                        for kb in range(nkb):
                            tkb = b * S + kb * 128
                            r = kb - qj * (SC // 128)
                            off = 128 * r if r > 0 else 0
                            ps = ps2s.tile([128, 2 * SC], F32, tag="ps",
                                           name=f"ps{kb}")
                            for hh in range(2):
                                nc.tensor.matmul(
                                    ps[:, hh * SC + off:(hh + 1) * SC],
                                    kT[:, tkb:tkb + 128],
                                    qT[:, (h0 + hh) * T + g * SC + off:
                                       (h0 + hh) * T + (g + 1) * SC],
                                    start=True, stop=True)
                            es = p2s.tile([128, 2 * SC], CDT, tag="es",
                                          name=f"es{kb}")
                            if off:
                                ps_ap = ps[:].rearrange(
                                    "p (h q) -> p h q", h=2)[:, :, off:]
                                es_ap = es[:].rearrange(
                                    "p (h q) -> p h q", h=2)[:, :, off:]
                            else:
                                ps_ap = ps[:]
                                es_ap = es[:]
                            nc.scalar.activation(es_ap, ps_ap, EXP,
                                                 scale=SCALE)
                            ess.append((es, tkb, off))
                            # PV/esum one block behind; p3 filler keeps the
                            # PE busy while ScalarE paces the exp stream
                            if kb >= 1:
                                emit_pv(kb - 1)
                                emit_esum(kb - 1)
                            if r >= 0:
                                # zero the causal triangle (queries
                                # [128r, 128r+128)) multiplicatively after
                                # exp; rides DVE a block ahead of its PV
                                roff = 128 * r
                                for hh in range(2):
                                    nc.vector.tensor_tensor(
                                        es[:, hh * SC + roff:
                                           hh * SC + roff + 128],
                                        es[:, hh * SC + roff:
                                           hh * SC + roff + 128],
                                        mk[:, r * SC + roff:
                                           r * SC + roff + 128], MULT)
                            if p3_eligible(g):
                                emit_p3_unit()
                        emit_pv(nkb - 1)
                        emit_esum(nkb - 1)
                        pz = ps2s.tile([128, 2 * SC], F32, tag="ps",
                                       name="pz")
                        for hh in range(2):
                            nc.tensor.matmul(
                                pz[:, hh * SC:(hh + 1) * SC], ones[:],
                                esum[:, hh * SC:(hh + 1) * SC],
                                start=True, stop=True)
                        zr = p2z.tile([128, 2 * SC], F32, tag="zr")
                        nc.vector.reciprocal_approx_fast(zr[:], pz[:])
                        ot = p2z.tile([128, 2 * SC], CDT, tag="ot")
                        nc.vector.tensor_tensor(ot[:], po[:], zr[:], MULT)
                        nc.sync.dma_start(
                            out=outT_t[g * HPC + h0],
                            in_=ot[:, 0:SC])
                        nc.sync.dma_start(
                            out=outT_t[g * HPC + h0 + 1],
                            in_=ot[:, SC:2 * SC])
                        # boundary filler covers the Z->recip->ot drain
                        # before the next block's first PV needs po
                        for _ in range(3):
                            if p3_eligible(g):
                                emit_p3_unit()
                    nc.gpsimd.collective_compute(
                        "AllGather", mybir.AluOpType.bypass,
                        replica_groups=[list(range(NCORES))],
                        ins=[outT_d[g * HPC:(g + 1) * HPC, :].opt()],
                        outs=[ag_d[g].opt()])
                # drain the remaining out-projection work with deeper
                # po3 buffering (attention PSUM pools are closed first)
                ps2ctx[1].__exit__(None, None, None)
                ps2ctx[0].__exit__(None, None, None)
                with tc.tile_pool(name="ps3g", bufs=3,
                                  space="PSUM") as ps3g:
                    p3_state["pool"] = ps3g
                    while p3_state["cursor"] < len(p3_units):
                        emit_p3_unit()

    nc.compile()
    return nc


def _run(inputs, trace=False, tmpdir=None):
    from concourse.bass_utils import run_bass_kernel_spmd

    if "nc" not in _CACHE:
        _CACHE["nc"] = build_nc()
    nc = _CACHE["nc"]
    shared, cores = host_prepare(
        inputs["x"], inputs["cos"], inputs["sin"], inputs["mask"],
        inputs["wq"], inputs["wk"], inputs["wv"], inputs["wo"])
    in_maps = []
    for i in range(NCORES):
        m = dict(shared)
        m.update(cores[i])
        in_maps.append(m)
    res = run_bass_kernel_spmd(nc, in_maps, list(range(NCORES)), trace=trace,
                               tmpdir=tmpdir)
    outs = []
    for i in range(NCORES):
        o = np.asarray(res.results[i]["out"], dtype=np.float32)
        # rows: (c/128)*NT + t-chunk, each 128*SC -> (CQ, T) -> (T, CQ)
        o = o.reshape(CQ // 128, NT, 128, SC).transpose(0, 2, 1, 3)
        outs.append(o.reshape(CQ, T).T)
    full = np.concatenate(outs, axis=1).reshape(B, S, D)
    return full, res


def kernel(**inputs):
    out, _ = _run(inputs, trace=False)
    return out.astype(np.float32)



# revision 32
# speedup vs baseline: 1.0605x; 1.0605x over previous
"""Trainium2 8-core GQA causal attention kernel (Bass/Tile).

Problem: B=2, S=2048, D=4096, 32 Q heads / 8 KV heads, HD=128, RoPE
(interleaved pairs), causal mask, output projection.

Sharding: 8-way tensor parallel over KV-head groups. Core i owns query
heads 4i..4i+3 (wq cols i*512..), kv head i (wk/wv cols i*128..), and
OUTPUT columns i*512.. of wo.  Per core (all in transposed layout; the
host passes xT and tile-packed weights so every DMA is contiguous):
  qT = wq_i.T @ x.T ; kT = wk_i.T @ x.T ; vT = wv_i.T @ x.T
  RoPE: z*cosF + (Pswap z)*sinF2  (pair swap via PE permutation matmul)
  v -> token-major via PE transpose
  S^T[tk,tq] = kT_tile.T @ qT_chunk (+mask on diagonal blocks)
  es = exp(S^T * scale) fused on ScalarE (PSUM->SBUF)
  out^T[c,tq] += v_tile @ es ; Z[tq] += ones @ es (replicated col-sums)
  out^T = out^T / Z  -> outT chunk (bf16)
AllGather (4 token-range chunks, overlapped with attention) -> attnT;
outP = wo_i.T @ attnT  (512 out cols, T); host concatenates + transposes.
"""
import sys
import numpy as np

sys.path.insert(0, "/opt/trn_rl_repo")

import ml_dtypes  # noqa: E402

BF16 = ml_dtypes.bfloat16

NCORES = 8
B, S, D = 2, 2048, 4096
H, KV, HD = 32, 8, 128
T = B * S
HPC = H // NCORES          # 4 query heads per core
CQ = HPC * HD              # 512
CKV = HD                   # 128
SC = 512                   # token chunk (free dim of moving operands)
ND = D // 128              # 32 contraction chunks
NT = T // SC               # 8 token chunks
NA = NCORES * CQ // 128    # 32 attention-dim chunks in phase 3
NAG = 4                    # all-gather chunks (2 token chunks each)
SCALE = float(HD) ** -0.5


def _pack(a, width):
    """(n*128, width) -> (n, 128*width) tile-contiguous rows."""
    n = a.shape[0] // 128
    return np.ascontiguousarray(a.reshape(n, 128, width).reshape(n, 128 * width))


def host_prepare(x, cos, sin, mask, wq, wk, wv, wo):
    xM = np.ascontiguousarray(np.asarray(x, dtype=np.float32).reshape(T, D))
    xT = np.ascontiguousarray(xM.T).astype(BF16)                 # (D, T)
    # xTp[d*NT+t] = tile (d-chunk, t-chunk) flattened (128, SC)
    xTp = np.ascontiguousarray(
        xT.reshape(ND, 128, NT, SC).transpose(0, 2, 1, 3)
    ).reshape(ND * NT, 128 * SC)
    cosF = np.repeat(np.asarray(cos, dtype=np.float32).T, 2, axis=0).astype(BF16)
    sinF2 = np.repeat(np.asarray(sin, dtype=np.float32).T, 2, axis=0)
    sinF2[0::2] *= -1.0
    sinF2 = sinF2.astype(BF16)                                   # (128, S)
    pswap = np.zeros((128, 128), dtype=np.float32)
    idx = np.arange(0, 128, 2)
    pswap[idx, idx + 1] = 1.0
    pswap[idx + 1, idx] = 1.0
    pswapT = pswap.astype(BF16)
    ident = np.eye(128, dtype=np.float32).astype(BF16)
    ones = np.ones((128, 128), dtype=np.float32).astype(BF16)
    # 0/1 keep-mask (bf16), applied multiplicatively AFTER exp
    maskT4 = np.concatenate(
        [(np.asarray(mask, dtype=np.float32)[0:SC, r * 128:(r + 1) * 128].T
          == 0.0).astype(np.float32)
         for r in range(4)], axis=1
    ).astype(BF16)                                               # (128, 4*SC)
    shared = dict(xT=xTp, cosF=cosF, sinF2=sinF2, pswapT=pswapT, ident=ident,
                  ones=ones, maskT4=maskT4)
    cores = []
    for i in range(NCORES):
        cores.append(dict(
            wq=_pack(np.ascontiguousarray(wq[:, i * CQ:(i + 1) * CQ]).astype(BF16), CQ),
            wk=_pack(np.ascontiguousarray(wk[:, i * CKV:(i + 1) * CKV]).astype(BF16), CKV),
            wv=_pack(np.ascontiguousarray(wv[:, i * CKV:(i + 1) * CKV]).astype(BF16), CKV),
            wo=_pack(np.ascontiguousarray(wo[:, i * CQ:(i + 1) * CQ]).astype(BF16), CQ),
        ))
    return shared, cores


_CACHE = {}


def build_nc():
    from concourse import bacc, mybir, tile

    F32 = mybir.dt.float32
    CDT = mybir.dt.bfloat16
    ADD = mybir.AluOpType.add
    DIV = mybir.AluOpType.divide
    MULT = mybir.AluOpType.mult
    EXP = mybir.ActivationFunctionType.Exp
    COPY = mybir.ActivationFunctionType.Copy

    nc = bacc.Bacc("TRN2", target_bir_lowering=False, debug=False,
                   num_devices=NCORES)

    def par(name, shape, dt, out=False):
        return nc.dram_tensor(name, shape, dt,
                              kind="ExternalOutput" if out else "ExternalInput").ap()

    xT_p = par("xT", [ND * NT, 128 * SC], CDT)
    wq_p = par("wq", [ND, 128 * CQ], CDT)
    wk_p = par("wk", [ND, 128 * CKV], CDT)
    wv_p = par("wv", [ND, 128 * CKV], CDT)
    wo_p = par("wo", [ND, 128 * CQ], CDT)
    cos_p = par("cosF", [HD, S], CDT)
    sin_p = par("sinF2", [HD, S], CDT)
    psw_p = par("pswapT", [128, 128], CDT)
    idn_p = par("ident", [128, 128], CDT)
    one_p = par("ones", [128, 128], CDT)
    msk_p = par("maskT4", [128, 4 * SC], CDT)
    # output: outP[c, t] packed as [(c/128)*NT + t-chunk, 128*SC]
    out_p = par("out", [(CQ // 128) * NT, 128 * SC], F32, out=True)

    xT_t = xT_p.rearrange("n (p c) -> n p c", p=128)
    out_t = out_p.rearrange("n (p c) -> n p c", p=128)

    with tile.TileContext(nc) as tc:
        with tc.tile_pool(name="res", bufs=1) as res, \
             tc.tile_pool(name="dram", bufs=1, space="DRAM") as dram:
            kT = res.tile([128, T], CDT, tag="kT")
            vsb = res.tile([128, T], CDT, tag="vsb")
            qT = res.tile([128, HPC * T], CDT, tag="qT")
            mk = res.tile([128, 4 * SC], CDT, tag="mk")
            ones = res.tile([128, 128], CDT, tag="ones")
            wos = res.tile([128, ND * CQ], CDT, tag="wos")
            nc.sync.dma_start(out=ones[:], in_=one_p[:])
            # packed outT rows: row = g*HPC + h
            outT_d = dram.tile([NT * HPC, 128 * SC], CDT, tag="outT")
            ag_d = [dram.tile([NCORES * HPC, 128 * SC], CDT, tag=f"ag{g}",
                              name=f"ag{g}", addr_space="Shared")
                    for g in range(NT)]

            # ---------------- phase 1: projections + rope + v transpose
            with tc.tile_pool(name="p1c", bufs=1) as p1c, \
                 tc.tile_pool(name="p1x", bufs=4) as p1x, \
                 tc.tile_pool(name="p1s", bufs=3) as p1s, \
                 tc.tile_pool(name="p1r", bufs=7) as p1r, \
                 tc.tile_pool(name="ps1", bufs=1, space="PSUM") as ps1, \
                 tc.tile_pool(name="ps1b", bufs=2, space="PSUM") as ps1b:
                wqs = p1c.tile([128, ND * CQ], CDT, tag="wqs")
                wks = p1c.tile([128, ND * CKV], CDT, tag="wks")
                wvs = p1c.tile([128, ND * CKV], CDT, tag="wvs")
                cosF = p1c.tile([128, S], CDT, tag="cosF")
                sinF2 = p1c.tile([128, S], CDT, tag="sinF2")
                psw = p1c.tile([128, 128], CDT, tag="psw")
                idn = p1c.tile([128, 128], CDT, tag="idn")
                # weights on the gpsimd queue so they don't block x tiles;
                # interleaved by d-chunk so d=0 of all three lands first.
                wq_t = wq_p.rearrange("n (p c) -> n p c", p=128)
                wk_t = wk_p.rearrange("n (p c) -> n p c", p=128)
                wv_t = wv_p.rearrange("n (p c) -> n p c", p=128)
                wo_t = wo_p.rearrange("n (p c) -> n p c", p=128)
                for d in range(ND):
                    nc.gpsimd.dma_start(out=wqs[:, d * CQ:(d + 1) * CQ],
                                        in_=wq_t[d])
                    nc.gpsimd.dma_start(out=wks[:, d * CKV:(d + 1) * CKV],
                                        in_=wk_t[d])
                    nc.gpsimd.dma_start(out=wvs[:, d * CKV:(d + 1) * CKV],
                                        in_=wv_t[d])
                    if d == 0:
                        nc.gpsimd.dma_start(out=cosF[:], in_=cos_p[:])
                        nc.gpsimd.dma_start(out=sinF2[:], in_=sin_p[:])
                # mask (phase 2) then wo (phase 3) trickle in behind the
                # projection weights on the gpsimd queue.
                nc.gpsimd.dma_start(out=mk[:], in_=msk_p[:])
                for d in range(ND):
                    nc.gpsimd.dma_start(out=wos[:, d * CQ:(d + 1) * CQ],
                                        in_=wo_t[d])

                for tcn in range(NT):
                    t0 = tcn * SC
                    s0 = (tcn % (S // SC)) * SC
                    pq = [ps1.tile([128, SC], F32, tag=f"pq{h}", name=f"pq{h}")
                          for h in range(HPC)]
                    pk = ps1.tile([128, SC], F32, tag="pk")
                    pv = ps1.tile([128, SC], F32, tag="pv")
                    for d in range(ND):
                        xt = p1x.tile([128, SC], CDT, tag="xt")
                        nc.sync.dma_start(out=xt[:], in_=xT_t[d * NT + tcn])
                        if tcn == 0 and d == 1:
                            # small constants ride behind the first x tiles
                            # so the leading matmuls aren't delayed
                            nc.sync.dma_start(out=psw[:], in_=psw_p[:])
                            nc.sync.dma_start(out=idn[:], in_=idn_p[:])
                        st = (d == 0)
                        sp = (d == ND - 1)
                        for h in range(HPC):
                            nc.tensor.matmul(
                                pq[h][:],
                                wqs[:, d * CQ + h * 128:d * CQ + (h + 1) * 128],
                                xt[:], start=st, stop=sp)
                        nc.tensor.matmul(pk[:],
                                         wks[:, d * CKV:(d + 1) * CKV],
                                         xt[:], start=st, stop=sp)
                        nc.tensor.matmul(pv[:],
                                         wvs[:, d * CKV:(d + 1) * CKV],
                                         xt[:], start=st, stop=sp)

                    # evict all psums first (frees banks for next chunk);
                    # alternate ScalarE/VectorE so the eviction chain drains
                    # at twice the single-engine rate (RoPE psh matmuls wait
                    # on these).
                    raws = []
                    for z in range(HPC + 2):
                        src = pq[z] if z < HPC else (pk if z == HPC else pv)
                        raw = p1r.tile([128, SC], CDT, tag="raw",
                                       name=f"raw{z}")
                        if z % 2 == 0:
                            nc.scalar.activation(raw[:], src[:], COPY)
                        else:
                            nc.vector.tensor_scalar_add(raw[:], src[:], 0.0)
                        raws.append(raw)
                    # rope (q heads + k)
                    for z in range(HPC + 1):
                        raw = raws[z]
                        psh = ps1b.tile([128, SC], F32, tag="psx", name="psh")
                        nc.tensor.matmul(psh[:], psw[:], raw[:],
                                         start=True, stop=True)
                        t1 = p1s.tile([128, SC], CDT, tag="t1")
                        nc.vector.tensor_tensor(t1[:], raw[:],
                                                cosF[:, s0:s0 + SC], MULT)
                        t2 = p1s.tile([128, SC], CDT, tag="t2")
                        nc.vector.tensor_tensor(t2[:], psh[:],
                                                sinF2[:, s0:s0 + SC], MULT)
                        if z < HPC:
                            dst = qT[:, z * T + t0:z * T + t0 + SC]
                        else:
                            dst = kT[:, t0:t0 + SC]
                        nc.vector.tensor_tensor(dst, t1[:], t2[:], ADD)
                    # v -> token-major via PE transpose
                    vts = raws[HPC + 1]
                    for j in range(SC // 128):
                        ptr = ps1b.tile([128, 128], CDT, tag="psx", name="ptr")
                        nc.tensor.transpose(ptr[:],
                                            vts[:, j * 128:(j + 1) * 128],
                                            idn[:])
                        nc.scalar.activation(
                            vsb[:, t0 + j * 128:t0 + (j + 1) * 128],
                            ptr[:], COPY)

            # ---------------- phase 2+3: attention + interleaved out-proj
            # Head-pair-wide layout: scores/es/po tiles cover 2 heads
            # ([128, 2*SC]). Z comes from a VectorE-accumulated esum plus a
            # single ones-matmul per head. The causal mask is applied
            # multiplicatively (0/1, bf16) AFTER exp on GpSimd, keeping the
            # scores->exp chain a pure PE->ScalarE path. Diagonal blocks are
            # trimmed: key block r only touches queries >= 128*r.
            # Phase-3 (out-projection) work is consumed as PE filler inside
            # attention once its AllGather chunk has landed, then drained.
            outT_t = outT_d.rearrange("n (p c) -> n p c", p=128)
            ags = [d.rearrange("n (p c) -> n p c", p=128) for d in ag_d]
            with tc.tile_pool(name="p2s", bufs=8) as p2s, \
                 tc.tile_pool(name="p2e", bufs=2) as p2e, \
                 tc.tile_pool(name="p2z", bufs=2) as p2z, \
                 tc.tile_pool(name="p3a", bufs=12) as p3a, \
                 tc.tile_pool(name="p3o", bufs=3) as p3o, \
                 tc.tile_pool(name="ps3f", bufs=1, space="PSUM") as ps3f:
                # phase-3 units: (g, half, a); two out-col blocks per half.
                p3_units = [(gp, half, a) for gp in range(NT)
                            for half in range(2) for a in range(NA)]
                p3_state = dict(cursor=0, po3=None)
                # simulated-time anchors (ms): start-of-block estimates used
                # to stop the scheduler from hoisting filler work ahead of
                # its AllGather (hoisted units stall the in-order PE/sync
                # queues on the unfinished collective).
                simt = [0.40]
                for g2 in range(NT):
                    qj2 = g2 % (S // SC)
                    simt.append(simt[-1] + 2 * 4 * (qj2 + 1) * 0.0011
                                + 0.004)

                def p3_eligible(gcur):
                    if p3_state["cursor"] >= len(p3_units):
                        return False
                    # wide margin: a filler matmul stuck on a late
                    # AllGather would block the whole PE stream
                    return gcur >= p3_units[p3_state["cursor"]][0] + 4

                def emit_p3_unit():
                    gp, half, a = p3_units[p3_state["cursor"]]
                    with tc.tile_wait_until(simt[min(gp + 3, NT)]):
                        _emit_p3_unit_inner(gp, half, a)
                    p3_state["cursor"] += 1

                def _emit_p3_unit_inner(gp, half, a):
                    if a == 0:
                        pool = p3_state.get("pool") or ps3f
                        p3_state["po3"] = pool.tile([128, 2 * SC], F32,
                                                    tag="po3", name="po3")
                    po3 = p3_state["po3"]
                    core, h = divmod(a, HPC)
                    at = p3a.tile([128, SC], CDT, tag="at")
                    if p3_state.get("pool") is not None:
                        # drain phase: the sync sequencer (~565ns per DMA
                        # trigger) would pace the 2-matmul units; spread
                        # triggers across otherwise-idle engine queues
                        eng = (nc.sync, nc.scalar, nc.gpsimd)[a % 3]
                    else:
                        eng = nc.sync
                    eng.dma_start(out=at[:], in_=ags[gp][core * HPC + h])
                    st = (a == 0)
                    sp = (a == NA - 1)
                    for j in range(2):
                        c = 2 * half + j
                        nc.tensor.matmul(
                            po3[:, j * SC:(j + 1) * SC],
                            wos[:, a * CQ + c * 128:a * CQ + (c + 1) * 128],
                            at[:], start=st, stop=sp)
                    if a == NA - 1:
                        # evict the two out-col blocks, split across
                        # ScalarE/VectorE to balance engine load
                        ob = p3o.tile([128, 2 * SC], F32, tag="ob")
                        nc.scalar.activation(ob[:, 0:SC], po3[:, 0:SC],
                                             COPY)
                        nc.vector.tensor_scalar_add(ob[:, SC:2 * SC],
                                                    po3[:, SC:2 * SC], 0.0)
                        for j in range(2):
                            c = 2 * half + j
                            nc.sync.dma_start(out=out_t[c * NT + gp],
                                              in_=ob[:, j * SC:(j + 1) * SC])

                ps2ctx = [tc.tile_pool(name="ps2s", bufs=2, space="PSUM"),
                          tc.tile_pool(name="ps2o", bufs=1, space="PSUM")]
                ps2s = ps2ctx[0].__enter__()
                ps2o = ps2ctx[1].__enter__()
                for g in range(NT):
                    b, qj = divmod(g, S // SC)
                    nkb = (SC // 128) * (qj + 1)
                    for hp in range(HPC // 2):
                        h0 = 2 * hp
                        po = ps2o.tile([128, 2 * SC], F32, tag="po",
                                       name="po")
                        esum = p2e.tile([128, 2 * SC], CDT, tag="esum",
                                        name="esum")
                        ess = []

                        def emit_pv(kb):
                            es, tkb, off = ess[kb]
                            st = (kb == 0)
                            sp = (kb == nkb - 1)
                            for hh in range(2):
                                nc.tensor.matmul(
                                    po[:, hh * SC + off:(hh + 1) * SC],
                                    vsb[:, tkb:tkb + 128],
                                    es[:, hh * SC + off:(hh + 1) * SC],
                                    start=st, stop=sp)

                        def emit_esum(kb):
                            # trails one block behind exp so the DVE never
                            # head-of-line blocks on exp(kb)
                            es, tkb, off = ess[kb]
                            if kb == 0:
                                nc.vector.tensor_scalar_add(
                                    esum[:], es[:], 0.0)
                            else:
                                es_ap = (es[:].rearrange(
                                    "p (h q) -> p h q", h=2)[:, :, off:]
                                    if off else es[:])
                                em_ap = (esum[:].rearrange(
                                    "p (h q) -> p h q", h=2)[:, :, off:]
                                    if off else esum[:])
                                nc.vector.tensor_tensor(
                                    em_ap, em_ap, es_ap, ADD)

                        for kb in range(nkb):
                            tkb = b * S + kb * 128
                            r = kb - qj * (SC // 128)
                            off = 128 * r if r > 0 else 0
                            ps = ps2s.tile([128, 2 * SC], F32, tag="ps",
                                           name=f"ps{kb}")
                            for hh in range(2):
                                nc.tensor.matmul(
                                    ps[:, hh * SC + off:(hh + 1) * SC],
                                    kT[:, tkb:tkb + 128],
                                    qT[:, (h0 + hh) * T + g * SC + off:
                                       (h0 + hh) * T + (g + 1) * SC],
                                    start=True, stop=True)
                            es = p2s.tile([128, 2 * SC], CDT, tag="es",
                                          name=f"es{kb}")
                            if off:
                                ps_ap = ps[:].rearrange(
                                    "p (h q) -> p h q", h=2)[:, :, off:]
                                es_ap = es[:].rearrange(
                                    "p (h q) -> p h q", h=2)[:, :, off:]
                            else:
                                ps_ap = ps[:]
                                es_ap = es[:]
                            nc.scalar.activation(es_ap, ps_ap, EXP,
                                                 scale=SCALE)
                            ess.append((es, tkb, off))
                            # PV/esum one block behind; p3 filler keeps the
                            # PE busy while ScalarE paces the exp stream
                            if kb >= 1:
                                emit_pv(kb - 1)
                                emit_esum(kb - 1)
                            if r >= 0:
                                # zero the causal triangle (queries
                                # [128r, 128r+128)) multiplicatively after
                                # exp; rides DVE a block ahead of its PV
                                roff = 128 * r
                                for hh in range(2):
                                    nc.vector.tensor_tensor(
                                        es[:, hh * SC + roff:
                                           hh * SC + roff + 128],
                                        es[:, hh * SC + roff:
                                           hh * SC + roff + 128],
                                        mk[:, r * SC + roff:
                                           r * SC + roff + 128], MULT)
                            if p3_eligible(g):
                                emit_p3_unit()
                        emit_pv(nkb - 1)
                        emit_esum(nkb - 1)
                        pz = ps2s.tile([128, 2 * SC], F32, tag="ps",
                                       name="pz")
                        for hh in range(2):
                            nc.tensor.matmul(
                                pz[:, hh * SC:(hh + 1) * SC], ones[:],
                                esum[:, hh * SC:(hh + 1) * SC],
                                start=True, stop=True)
                        zr = p2z.tile([128, 2 * SC], F32, tag="zr")
                        nc.vector.reciprocal_approx_fast(zr[:], pz[:])
                        ot = p2z.tile([128, 2 * SC], CDT, tag="ot")
                        nc.vector.tensor_tensor(ot[:], po[:], zr[:], MULT)
                        nc.sync.dma_start(
                            out=outT_t[g * HPC + h0],
                            in_=ot[:, 0:SC])
                        nc.sync.dma_start(
                            out=outT_t[g * HPC + h0 + 1],
                            in_=ot[:, SC:2 * SC])
                        # boundary filler covers the Z->recip->ot drain
                        # before the next block's first PV needs po
                        for _ in range(3):
                            if p3_eligible(g):
                                emit_p3_unit()
                    nc.gpsimd.collective_compute(
                        "AllGather", mybir.AluOpType.bypass,
                        replica_groups=[list(range(NCORES))],
                        ins=[outT_d[g * HPC:(g + 1) * HPC, :].opt()],
                        outs=[ag_d[g].opt()])
                # drain the remaining out-projection work with deeper
                # po3 buffering (attention PSUM pools are closed first)
                ps2ctx[1].__exit__(None, None, None)
                ps2ctx[0].__exit__(None, None, None)
                with tc.tile_pool(name="ps3g", bufs=3,
                                  space="PSUM") as ps3g:
                    p3_state["pool"] = ps3g
                    while p3_state["cursor"] < len(p3_units):
                        emit_p3_unit()

    nc.compile()
    return nc


def _run(inputs, trace=False, tmpdir=None):
    from concourse.bass_utils import run_bass_kernel_spmd

    if "nc" not in _CACHE:
        _CACHE["nc"] = build_nc()
    nc = _CACHE["nc"]
    shared, cores = host_prepare(
        inputs["x"], inputs["cos"], inputs["sin"], inputs["mask"],
        inputs["wq"], inputs["wk"], inputs["wv"], inputs["wo"])
    in_maps = []
    for i in range(NCORES):
        m = dict(shared)
        m.update(cores[i])
        in_maps.append(m)
    res = run_bass_kernel_spmd(nc, in_maps, list(range(NCORES)), trace=trace,
                               tmpdir=tmpdir)
    outs = []
    for i in range(NCORES):
        o = np.asarray(res.results[i]["out"], dtype=np.float32)
        # rows: (c/128)*NT + t-chunk, each 128*SC -> (CQ, T) -> (T, CQ)
        o = o.reshape(CQ // 128, NT, 128, SC).transpose(0, 2, 1, 3)
        outs.append(o.reshape(CQ, T).T)
    full = np.concatenate(outs, axis=1).reshape(B, S, D)
    return full, res


def kernel(**inputs):
    out, _ = _run(inputs, trace=False)
    return out.astype(np.float32)



# revision 35
# speedup vs baseline: 1.0760x; 1.0146x over previous
"""Trainium2 8-core GQA causal attention kernel (Bass/Tile).

Problem: B=2, S=2048, D=4096, 32 Q heads / 8 KV heads, HD=128, RoPE
(interleaved pairs), causal mask, output projection.

Sharding: 8-way tensor parallel over KV-head groups. Core i owns query
heads 4i..4i+3 (wq cols i*512..), kv head i (wk/wv cols i*128..), and
OUTPUT columns i*512.. of wo.  Per core (all in transposed layout; the
host passes xT and tile-packed weights so every DMA is contiguous):
  qT = wq_i.T @ x.T ; kT = wk_i.T @ x.T ; vT = wv_i.T @ x.T
  RoPE: z*cosF + (Pswap z)*sinF2  (pair swap via PE permutation matmul)
  v -> token-major via PE transpose
  S^T[tk,tq] = kT_tile.T @ qT_chunk (+mask on diagonal blocks)
  es = exp(S^T * scale) fused on ScalarE (PSUM->SBUF)
  out^T[c,tq] += v_tile @ es ; Z[tq] += ones @ es (replicated col-sums)
  out^T = out^T / Z  -> outT chunk (bf16)
AllGather (4 token-range chunks, overlapped with attention) -> attnT;
outP = wo_i.T @ attnT  (512 out cols, T); host concatenates + transposes.
"""
import sys
import numpy as np

sys.path.insert(0, "/opt/trn_rl_repo")

import ml_dtypes  # noqa: E402

BF16 = ml_dtypes.bfloat16

NCORES = 8
B, S, D = 2, 2048, 4096
H, KV, HD = 32, 8, 128
T = B * S
HPC = H // NCORES          # 4 query heads per core
CQ = HPC * HD              # 512
CKV = HD                   # 128
SC = 512                   # token chunk (free dim of moving operands)
ND = D // 128              # 32 contraction chunks
NT = T // SC               # 8 token chunks
NA = NCORES * CQ // 128    # 32 attention-dim chunks in phase 3
NAG = 4                    # all-gather chunks (2 token chunks each)
SCALE = float(HD) ** -0.5


def _pack(a, width):
    """(n*128, width) -> (n, 128*width) tile-contiguous rows."""
    n = a.shape[0] // 128
    return np.ascontiguousarray(a.reshape(n, 128, width).reshape(n, 128 * width))


def host_prepare(x, cos, sin, mask, wq, wk, wv, wo):
    xM = np.ascontiguousarray(np.asarray(x, dtype=np.float32).reshape(T, D))
    xT = np.ascontiguousarray(xM.T).astype(BF16)                 # (D, T)
    # xTp[d*NT+t] = tile (d-chunk, t-chunk) flattened (128, SC)
    xTp = np.ascontiguousarray(
        xT.reshape(ND, 128, NT, SC).transpose(0, 2, 1, 3)
    ).reshape(ND * NT, 128 * SC)
    cosF = np.repeat(np.asarray(cos, dtype=np.float32).T, 2, axis=0).astype(BF16)
    sinF2 = np.repeat(np.asarray(sin, dtype=np.float32).T, 2, axis=0)
    sinF2[0::2] *= -1.0
    sinF2 = sinF2.astype(BF16)                                   # (128, S)
    pswap = np.zeros((128, 128), dtype=np.float32)
    idx = np.arange(0, 128, 2)
    pswap[idx, idx + 1] = 1.0
    pswap[idx + 1, idx] = 1.0
    pswapT = pswap.astype(BF16)
    ident = np.eye(128, dtype=np.float32).astype(BF16)
    ones = np.ones((128, 128), dtype=np.float32).astype(BF16)
    # 0/1 keep-mask (bf16), applied multiplicatively AFTER exp
    maskT4 = np.concatenate(
        [(np.asarray(mask, dtype=np.float32)[0:SC, r * 128:(r + 1) * 128].T
          == 0.0).astype(np.float32)
         for r in range(4)], axis=1
    ).astype(BF16)                                               # (128, 4*SC)
    shared = dict(xT=xTp, cosF=cosF, sinF2=sinF2, pswapT=pswapT, ident=ident,
                  ones=ones, maskT4=maskT4)
    cores = []
    for i in range(NCORES):
        cores.append(dict(
            wq=_pack(np.ascontiguousarray(wq[:, i * CQ:(i + 1) * CQ]).astype(BF16), CQ),
            wk=_pack(np.ascontiguousarray(wk[:, i * CKV:(i + 1) * CKV]).astype(BF16), CKV),
            wv=_pack(np.ascontiguousarray(wv[:, i * CKV:(i + 1) * CKV]).astype(BF16), CKV),
            wo=_pack(np.ascontiguousarray(wo[:, i * CQ:(i + 1) * CQ]).astype(BF16), CQ),
        ))
    return shared, cores


_CACHE = {}


def build_nc():
    from concourse import bacc, mybir, tile

    F32 = mybir.dt.float32
    CDT = mybir.dt.bfloat16
    ADD = mybir.AluOpType.add
    DIV = mybir.AluOpType.divide
    MULT = mybir.AluOpType.mult
    EXP = mybir.ActivationFunctionType.Exp
    COPY = mybir.ActivationFunctionType.Copy

    nc = bacc.Bacc("TRN2", target_bir_lowering=False, debug=False,
                   num_devices=NCORES)

    def par(name, shape, dt, out=False):
        return nc.dram_tensor(name, shape, dt,
                              kind="ExternalOutput" if out else "ExternalInput").ap()

    xT_p = par("xT", [ND * NT, 128 * SC], CDT)
    wq_p = par("wq", [ND, 128 * CQ], CDT)
    wk_p = par("wk", [ND, 128 * CKV], CDT)
    wv_p = par("wv", [ND, 128 * CKV], CDT)
    wo_p = par("wo", [ND, 128 * CQ], CDT)
    cos_p = par("cosF", [HD, S], CDT)
    sin_p = par("sinF2", [HD, S], CDT)
    psw_p = par("pswapT", [128, 128], CDT)
    idn_p = par("ident", [128, 128], CDT)
    one_p = par("ones", [128, 128], CDT)
    msk_p = par("maskT4", [128, 4 * SC], CDT)
    # output: outP[c, t] packed as [(c/128)*NT + t-chunk, 128*SC]
    out_p = par("out", [(CQ // 128) * NT, 128 * SC], CDT, out=True)

    xT_t = xT_p.rearrange("n (p c) -> n p c", p=128)
    out_t = out_p.rearrange("n (p c) -> n p c", p=128)

    with tile.TileContext(nc) as tc:
        with tc.tile_pool(name="res", bufs=1) as res, \
             tc.tile_pool(name="dram", bufs=1, space="DRAM") as dram:
            kT = res.tile([128, T], CDT, tag="kT")
            vsb = res.tile([128, T], CDT, tag="vsb")
            qT = res.tile([128, HPC * T], CDT, tag="qT")
            mk = res.tile([128, 4 * SC], CDT, tag="mk")
            ones = res.tile([128, 128], CDT, tag="ones")
            wos = res.tile([128, ND * CQ], CDT, tag="wos")
            nc.sync.dma_start(out=ones[:], in_=one_p[:])
            # packed outT rows: row = g*HPC + h
            outT_d = dram.tile([NT * HPC, 128 * SC], CDT, tag="outT")
            ag_d = [dram.tile([NCORES * HPC, 128 * SC], CDT, tag=f"ag{g}",
                              name=f"ag{g}", addr_space="Shared")
                    for g in range(NT)]

            # ---------------- phase 1: projections + rope + v transpose
            with tc.tile_pool(name="p1c", bufs=1) as p1c, \
                 tc.tile_pool(name="p1x", bufs=4) as p1x, \
                 tc.tile_pool(name="p1s", bufs=3) as p1s, \
                 tc.tile_pool(name="p1r", bufs=7) as p1r, \
                 tc.tile_pool(name="ps1", bufs=1, space="PSUM") as ps1, \
                 tc.tile_pool(name="ps1b", bufs=2, space="PSUM") as ps1b:
                wqs = p1c.tile([128, ND * CQ], CDT, tag="wqs")
                wks = p1c.tile([128, ND * CKV], CDT, tag="wks")
                wvs = p1c.tile([128, ND * CKV], CDT, tag="wvs")
                cosF = p1c.tile([128, S], CDT, tag="cosF")
                sinF2 = p1c.tile([128, S], CDT, tag="sinF2")
                psw = p1c.tile([128, 128], CDT, tag="psw")
                idn = p1c.tile([128, 128], CDT, tag="idn")
                # weights on the gpsimd queue so they don't block x tiles;
                # interleaved by d-chunk so d=0 of all three lands first.
                wq_t = wq_p.rearrange("n (p c) -> n p c", p=128)
                wk_t = wk_p.rearrange("n (p c) -> n p c", p=128)
                wv_t = wv_p.rearrange("n (p c) -> n p c", p=128)
                wo_t = wo_p.rearrange("n (p c) -> n p c", p=128)
                for d in range(ND):
                    nc.gpsimd.dma_start(out=wqs[:, d * CQ:(d + 1) * CQ],
                                        in_=wq_t[d])
                    nc.gpsimd.dma_start(out=wks[:, d * CKV:(d + 1) * CKV],
                                        in_=wk_t[d])
                    nc.gpsimd.dma_start(out=wvs[:, d * CKV:(d + 1) * CKV],
                                        in_=wv_t[d])
                    if d == 0:
                        nc.gpsimd.dma_start(out=cosF[:], in_=cos_p[:])
                        nc.gpsimd.dma_start(out=sinF2[:], in_=sin_p[:])
                # mask (phase 2) then wo (phase 3) trickle in behind the
                # projection weights on the gpsimd queue.
                nc.gpsimd.dma_start(out=mk[:], in_=msk_p[:])
                for d in range(ND):
                    nc.gpsimd.dma_start(out=wos[:, d * CQ:(d + 1) * CQ],
                                        in_=wo_t[d])

                for tcn in range(NT):
                    t0 = tcn * SC
                    s0 = (tcn % (S // SC)) * SC
                    pq = [ps1.tile([128, SC], F32, tag=f"pq{h}", name=f"pq{h}")
                          for h in range(HPC)]
                    pk = ps1.tile([128, SC], F32, tag="pk")
                    pv = ps1.tile([128, SC], F32, tag="pv")
                    for d in range(ND):
                        xt = p1x.tile([128, SC], CDT, tag="xt")
                        nc.sync.dma_start(out=xt[:], in_=xT_t[d * NT + tcn])
                        if tcn == 0 and d == 1:
                            # small constants ride behind the first x tiles
                            # so the leading matmuls aren't delayed
                            nc.sync.dma_start(out=psw[:], in_=psw_p[:])
                            nc.sync.dma_start(out=idn[:], in_=idn_p[:])
                        st = (d == 0)
                        sp = (d == ND - 1)
                        for h in range(HPC):
                            nc.tensor.matmul(
                                pq[h][:],
                                wqs[:, d * CQ + h * 128:d * CQ + (h + 1) * 128],
                                xt[:], start=st, stop=sp)
                        nc.tensor.matmul(pk[:],
                                         wks[:, d * CKV:(d + 1) * CKV],
                                         xt[:], start=st, stop=sp)
                        nc.tensor.matmul(pv[:],
                                         wvs[:, d * CKV:(d + 1) * CKV],
                                         xt[:], start=st, stop=sp)

                    # evict all psums first (frees banks for next chunk);
                    # alternate ScalarE/VectorE so the eviction chain drains
                    # at twice the single-engine rate (RoPE psh matmuls wait
                    # on these).
                    raws = []
                    for z in range(HPC + 2):
                        src = pq[z] if z < HPC else (pk if z == HPC else pv)
                        raw = p1r.tile([128, SC], CDT, tag="raw",
                                       name=f"raw{z}")
                        if z % 2 == 0:
                            nc.scalar.activation(raw[:], src[:], COPY)
                        else:
                            nc.vector.tensor_scalar_add(raw[:], src[:], 0.0)
                        raws.append(raw)
                    # rope (q heads + k)
                    for z in range(HPC + 1):
                        raw = raws[z]
                        psh = ps1b.tile([128, SC], F32, tag="psx", name="psh")
                        nc.tensor.matmul(psh[:], psw[:], raw[:],
                                         start=True, stop=True)
                        t1 = p1s.tile([128, SC], CDT, tag="t1")
                        nc.vector.tensor_tensor(t1[:], raw[:],
                                                cosF[:, s0:s0 + SC], MULT)
                        t2 = p1s.tile([128, SC], CDT, tag="t2")
                        nc.vector.tensor_tensor(t2[:], psh[:],
                                                sinF2[:, s0:s0 + SC], MULT)
                        if z < HPC:
                            dst = qT[:, z * T + t0:z * T + t0 + SC]
                        else:
                            dst = kT[:, t0:t0 + SC]
                        nc.vector.tensor_tensor(dst, t1[:], t2[:], ADD)
                    # v -> token-major via PE transpose
                    vts = raws[HPC + 1]
                    for j in range(SC // 128):
                        ptr = ps1b.tile([128, 128], CDT, tag="psx", name="ptr")
                        nc.tensor.transpose(ptr[:],
                                            vts[:, j * 128:(j + 1) * 128],
                                            idn[:])
                        nc.scalar.activation(
                            vsb[:, t0 + j * 128:t0 + (j + 1) * 128],
                            ptr[:], COPY)

            # ---------------- phase 2+3: attention + interleaved out-proj
            # Head-pair-wide layout: scores/es/po tiles cover 2 heads
            # ([128, 2*SC]). Z comes from a VectorE-accumulated esum plus a
            # single ones-matmul per head. The causal mask is applied
            # multiplicatively (0/1, bf16) AFTER exp on GpSimd, keeping the
            # scores->exp chain a pure PE->ScalarE path. Diagonal blocks are
            # trimmed: key block r only touches queries >= 128*r.
            # Phase-3 (out-projection) work is consumed as PE filler inside
            # attention once its AllGather chunk has landed, then drained.
            outT_t = outT_d.rearrange("n (p c) -> n p c", p=128)
            ags = [d.rearrange("n (p c) -> n p c", p=128) for d in ag_d]
            with tc.tile_pool(name="p2s", bufs=8) as p2s, \
                 tc.tile_pool(name="p2e", bufs=2) as p2e, \
                 tc.tile_pool(name="p2z", bufs=2) as p2z, \
                 tc.tile_pool(name="p3a", bufs=12) as p3a, \
                 tc.tile_pool(name="p3o", bufs=3) as p3o, \
                 tc.tile_pool(name="ps3f", bufs=1, space="PSUM") as ps3f:
                # phase-3 units: (g, half, a); two out-col blocks per half.
                p3_units = [(gp, half, a) for gp in range(NT)
                            for half in range(2) for a in range(NA)]
                p3_state = dict(cursor=0, po3=None)
                # simulated-time anchors (ms): start-of-block estimates used
                # to stop the scheduler from hoisting filler work ahead of
                # its AllGather (hoisted units stall the in-order PE/sync
                # queues on the unfinished collective).
                simt = [0.40]
                for g2 in range(NT):
                    qj2 = g2 % (S // SC)
                    simt.append(simt[-1] + 2 * 4 * (qj2 + 1) * 0.0011
                                + 0.004)

                def p3_eligible(gcur):
                    if p3_state["cursor"] >= len(p3_units):
                        return False
                    # wide margin: a filler matmul stuck on a late
                    # AllGather would block the whole PE stream
                    return gcur >= p3_units[p3_state["cursor"]][0] + 4

                def emit_p3_unit():
                    gp, half, a = p3_units[p3_state["cursor"]]
                    with tc.tile_wait_until(simt[min(gp + 3, NT)]):
                        _emit_p3_unit_inner(gp, half, a)
                    p3_state["cursor"] += 1

                def _emit_p3_unit_inner(gp, half, a):
                    if a == 0:
                        pool = p3_state.get("pool") or ps3f
                        p3_state["po3"] = pool.tile([128, 2 * SC], F32,
                                                    tag="po3", name="po3")
                    po3 = p3_state["po3"]
                    core, h = divmod(a, HPC)
                    at = p3a.tile([128, SC], CDT, tag="at")
                    if p3_state.get("pool") is not None:
                        # drain phase: the sync sequencer (~565ns per DMA
                        # trigger) would pace the 2-matmul units; spread
                        # triggers across otherwise-idle engine queues.
                        # ScalarE joins late (it drains attention exps at
                        # the transition).
                        p3_state["n"] = p3_state.get("n", 0) + 1
                        if p3_state["n"] < 64:
                            eng = (nc.sync, nc.gpsimd)[a % 2]
                        else:
                            eng = (nc.sync, nc.scalar, nc.gpsimd)[a % 3]
                    else:
                        eng = nc.sync
                    eng.dma_start(out=at[:], in_=ags[gp][core * HPC + h])
                    st = (a == 0)
                    sp = (a == NA - 1)
                    for j in range(2):
                        c = 2 * half + j
                        nc.tensor.matmul(
                            po3[:, j * SC:(j + 1) * SC],
                            wos[:, a * CQ + c * 128:a * CQ + (c + 1) * 128],
                            at[:], start=st, stop=sp)
                    if a == NA - 1:
                        # evict the two out-col blocks, split across
                        # ScalarE/VectorE to balance engine load
                        ob = p3o.tile([128, 2 * SC], CDT, tag="ob")
                        nc.scalar.activation(ob[:, 0:SC], po3[:, 0:SC],
                                             COPY)
                        nc.vector.tensor_scalar_add(ob[:, SC:2 * SC],
                                                    po3[:, SC:2 * SC], 0.0)
                        for j in range(2):
                            c = 2 * half + j
                            nc.sync.dma_start(out=out_t[c * NT + gp],
                                              in_=ob[:, j * SC:(j + 1) * SC])

                ps2ctx = [tc.tile_pool(name="ps2s", bufs=2, space="PSUM"),
                          tc.tile_pool(name="ps2o", bufs=1, space="PSUM")]
                ps2s = ps2ctx[0].__enter__()
                ps2o = ps2ctx[1].__enter__()
                for g in range(NT):
                    b, qj = divmod(g, S // SC)
                    nkb = (SC // 128) * (qj + 1)
                    for hp in range(HPC // 2):
                        h0 = 2 * hp
                        po = ps2o.tile([128, 2 * SC], F32, tag="po",
                                       name="po")
                        esum = p2e.tile([128, 2 * SC], CDT, tag="esum",
                                        name="esum")
                        ess = []

                        def emit_pv(kb):
                            es, tkb, off = ess[kb]
                            st = (kb == 0)
                            sp = (kb == nkb - 1)
                            for hh in range(2):
                                nc.tensor.matmul(
                                    po[:, hh * SC + off:(hh + 1) * SC],
                                    vsb[:, tkb:tkb + 128],
                                    es[:, hh * SC + off:(hh + 1) * SC],
                                    start=st, stop=sp)

                        def emit_esum(kb):
                            # trails one block behind exp so the DVE never
                            # head-of-line blocks on exp(kb)
                            es, tkb, off = ess[kb]
                            if kb == 0:
                                nc.vector.tensor_scalar_add(
                                    esum[:], es[:], 0.0)
                            else:
                                es_ap = (es[:].rearrange(
                                    "p (h q) -> p h q", h=2)[:, :, off:]
                                    if off else es[:])
                                em_ap = (esum[:].rearrange(
                                    "p (h q) -> p h q", h=2)[:, :, off:]
                                    if off else esum[:])
                                nc.vector.tensor_tensor(
                                    em_ap, em_ap, es_ap, ADD)

                        for kb in range(nkb):
                            tkb = b * S + kb * 128
                            r = kb - qj * (SC // 128)
                            off = 128 * r if r > 0 else 0
                            ps = ps2s.tile([128, 2 * SC], F32, tag="ps",
                                           name=f"ps{kb}")
                            for hh in range(2):
                                nc.tensor.matmul(
                                    ps[:, hh * SC + off:(hh + 1) * SC],
                                    kT[:, tkb:tkb + 128],
                                    qT[:, (h0 + hh) * T + g * SC + off:
                                       (h0 + hh) * T + (g + 1) * SC],
                                    start=True, stop=True)
                            es = p2s.tile([128, 2 * SC], CDT, tag="es",
                                          name=f"es{kb}")
                            if off:
                                ps_ap = ps[:].rearrange(
                                    "p (h q) -> p h q", h=2)[:, :, off:]
                                es_ap = es[:].rearrange(
                                    "p (h q) -> p h q", h=2)[:, :, off:]
                            else:
                                ps_ap = ps[:]
                                es_ap = es[:]
                            nc.scalar.activation(es_ap, ps_ap, EXP,
                                                 scale=SCALE)
                            ess.append((es, tkb, off))
                            # PV/esum one block behind; p3 filler keeps the
                            # PE busy while ScalarE paces the exp stream
                            if kb >= 1:
                                emit_pv(kb - 1)
                                emit_esum(kb - 1)
                            if r >= 0:
                                # zero the causal triangle (queries
                                # [128r, 128r+128)) multiplicatively after
                                # exp; rides DVE a block ahead of its PV
                                roff = 128 * r
                                for hh in range(2):
                                    nc.vector.tensor_tensor(
                                        es[:, hh * SC + roff:
                                           hh * SC + roff + 128],
                                        es[:, hh * SC + roff:
                                           hh * SC + roff + 128],
                                        mk[:, r * SC + roff:
                                           r * SC + roff + 128], MULT)
                            if p3_eligible(g):
                                emit_p3_unit()
                        emit_pv(nkb - 1)
                        emit_esum(nkb - 1)
                        pz = ps2s.tile([128, 2 * SC], F32, tag="ps",
                                       name="pz")
                        for hh in range(2):
                            nc.tensor.matmul(
                                pz[:, hh * SC:(hh + 1) * SC], ones[:],
                                esum[:, hh * SC:(hh + 1) * SC],
                                start=True, stop=True)
                        zr = p2z.tile([128, 2 * SC], F32, tag="zr")
                        nc.vector.reciprocal_approx_fast(zr[:], pz[:])
                        ot = p2z.tile([128, 2 * SC], CDT, tag="ot")
                        nc.vector.tensor_tensor(ot[:], po[:], zr[:], MULT)
                        nc.sync.dma_start(
                            out=outT_t[g * HPC + h0],
                            in_=ot[:, 0:SC])
                        nc.sync.dma_start(
                            out=outT_t[g * HPC + h0 + 1],
                            in_=ot[:, SC:2 * SC])
                        # boundary filler covers the Z->recip->ot drain
                        # before the next block's first PV needs po
                        for _ in range(3):
                            if p3_eligible(g):
                                emit_p3_unit()
                    nc.gpsimd.collective_compute(
                        "AllGather", mybir.AluOpType.bypass,
                        replica_groups=[list(range(NCORES))],
                        ins=[outT_d[g * HPC:(g + 1) * HPC, :].opt()],
                        outs=[ag_d[g].opt()])
                # drain the remaining out-projection work with deeper
                # po3 buffering (attention PSUM pools are closed first)
                ps2ctx[1].__exit__(None, None, None)
                ps2ctx[0].__exit__(None, None, None)
                with tc.tile_pool(name="ps3g", bufs=3,
                                  space="PSUM") as ps3g:
                    p3_state["pool"] = ps3g
                    while p3_state["cursor"] < len(p3_units):
                        emit_p3_unit()

    nc.compile()
    return nc


def _run(inputs, trace=False, tmpdir=None):
    from concourse.bass_utils import run_bass_kernel_spmd

    if "nc" not in _CACHE:
        _CACHE["nc"] = build_nc()
    nc = _CACHE["nc"]
    shared, cores = host_prepare(
        inputs["x"], inputs["cos"], inputs["sin"], inputs["mask"],
        inputs["wq"], inputs["wk"], inputs["wv"], inputs["wo"])
    in_maps = []
    for i in range(NCORES):
        m = dict(shared)
        m.update(cores[i])
        in_maps.append(m)
    res = run_bass_kernel_spmd(nc, in_maps, list(range(NCORES)), trace=trace,
                               tmpdir=tmpdir)
    outs = []
    for i in range(NCORES):
        o = np.asarray(res.results[i]["out"], dtype=np.float32)
        # rows: (c/128)*NT + t-chunk, each 128*SC -> (CQ, T) -> (T, CQ)
        o = o.reshape(CQ // 128, NT, 128, SC).transpose(0, 2, 1, 3)
        outs.append(o.reshape(CQ, T).T)
    full = np.concatenate(outs, axis=1).reshape(B, S, D)
    return full, res


def kernel(**inputs):
    out, _ = _run(inputs, trace=False)
    return out.astype(np.float32)

